# revision 21
# baseline (speedup 1.0000x reference)
"""Trainium2 Bass kernel for nn_DecoderLayer_90967407329666.

Decoder layer: LN1 -> QKV (+type emb) -> multi-axis RoPE -> causal SDPA
-> residual -> LN2 -> SwiGLU FFN -> residual.  B=2, T=2048, D=768, H=8,
DFF=2048, NTYPE=16, NAX=2 rotary axes of 32 dims each.

Sharding (8 cores):
  Phase 1 (token-parallel): core c owns 512 tokens (batch c//4, tokens
    512*(c%4)..). LN1 is folded into the projections: q/k/v come from
    matmuls on xs = x*alpha (bf16) plus a rank-1 mean-correction matmul
    (colsum(W) (x) beta) accumulated in PSUM; the b1 bias is folded into
    the type-emb tables host-side.
  One merged AllToAll moves a [2304, 512] bf16 slab (per head h: 96 rows
    q | 96 rows k | 96 rows v-token-major) so core c ends up with head c
    for all 4096 tokens.
  Phase 2 (head-parallel): causal attention per head, exp without max
    subtraction, score tiles paired into [128,1024] PSUM so one Exp
    covers two k-tiles, per-unit softmax tail straight out of PSUM.
  AllToAll #2 returns o (bf16) token-parallel.
  Phase 3: residual + folded LN2 + fused fc1+silu+fc2 stream + residual.
    W1/W2 are prefetched in bf16 at phase-1 start and stay resident.

All heavy matmuls run with bf16 operands (full PE rate); the residual
stream stays fp32; stats run in fp32r.
"""

import sys

sys.path.insert(0, "/opt/trn_rl_repo")

import numpy as np

import contextlib

import concourse.bacc as bacc
import concourse.bass as bass
import concourse.tile as tile
from concourse import mybir
from concourse.bass_utils import run_bass_kernel_spmd

# ---- problem constants (hardcoded per contest rules) ----
B, T = 2, 2048
D, H, DFF, NTYPE = 768, 8, 2048, 16
NAX = 2
HD = D // H            # 96
DR = HD // (NAX + 1)   # 32
EPS = 1e-5
THETA = 10000.0
N_CORES = 8
TPC = 512              # tokens per core
NSUP = 4               # supertiles per batch (2048/512)
KD = D // 128          # 6 contraction chunks over D
SCALE = 1.0 / np.sqrt(np.float32(HD))

F32 = mybir.dt.float32
F32R = mybir.dt.float32r
BF16 = mybir.dt.bfloat16
I32 = mybir.dt.int32

# Cody-Waite split of 2*pi (C1 has 12 mantissa bits -> n*C1 exact for n<2^11)
C1 = float(np.float32(np.floor(2 * np.pi * 2**9) / 2**9))
C2 = float(np.float32(2 * np.pi - C1))
C3 = float(np.float32(2 * np.pi - C1 - float(np.float32(2 * np.pi - C1))))
HALF_PI = float(np.pi / 2)

# qk output-feature permutation: 12 slices of 128 rows
#   slices 0..7  : [q_h dims 0:64 | k_h dims 0:64]   (rope rows)
#   slice  8, 9  : q tails (dims 64:96) of heads 0..3 / 4..7
#   slice 10,11  : k tails of heads 0..3 / 4..7
def _qk_colperm():
    cols = []
    for h in range(H):
        cols += list(range(96 * h, 96 * h + 64))          # q_h 0:64
        cols += list(range(768 + 96 * h, 768 + 96 * h + 64))  # k_h 0:64
    for h in range(H):
        cols += list(range(96 * h + 64, 96 * h + 96))     # q tails
    for h in range(H):
        cols += list(range(768 + 96 * h + 64, 768 + 96 * h + 96))  # k tails
    return np.array(cols)

QK_PERM = _qk_colperm()

_prog_cache = {}

# merged slab geometry: per head h, rows 288h+0:96 q | +96:192 k |
# +192:288 v (token-major inside)
BLK = 288


def build_program():
    key = 0
    if key in _prog_cache:
        return _prog_cache[key]
    nc = bacc.Bacc("TRN2", target_bir_lowering=False, debug=False,
                   num_devices=N_CORES)
    alu = mybir.AluOpType
    act = mybir.ActivationFunctionType

    # ---------------- DRAM I/O ----------------
    xT_d = nc.dram_tensor("xT", [D, TPC], F32, kind="ExternalInput")
    wqk_d = nc.dram_tensor("Wqk", [D, 1536], BF16, kind="ExternalInput")
    wv_d = nc.dram_tensor("Wv", [D, D], BF16, kind="ExternalInput")
    teq_d = nc.dram_tensor("te_q", [NTYPE, 1536], BF16, kind="ExternalInput")
    tek_d = nc.dram_tensor("te_k", [NTYPE, 1536], BF16, kind="ExternalInput")
    csqk_d = nc.dram_tensor("csqk", [1, 1536], BF16, kind="ExternalInput")
    csv_d = nc.dram_tensor("csv", [2, D], BF16, kind="ExternalInput")
    qtype_d = nc.dram_tensor("qtype", [1, TPC], F32R, kind="ExternalInput")
    ktype_d = nc.dram_tensor("ktype", [1, TPC], F32R, kind="ExternalInput")
    pos4_d = nc.dram_tensor("pos4", [4, TPC], F32R, kind="ExternalInput")
    invf_d = nc.dram_tensor("invf", [128, 1], F32, kind="ExternalInput")
    w1_d = nc.dram_tensor("W1", [D, 2 * DFF], BF16, kind="ExternalInput")
    cs1_d = nc.dram_tensor("cs1", [1, 2 * DFF], BF16, kind="ExternalInput")
    w2_d = nc.dram_tensor("W2", [DFF, D], BF16, kind="ExternalInput")
    b1a_d = nc.dram_tensor("b1a", [128, 16], F32, kind="ExternalInput")
    b1g_d = nc.dram_tensor("b1g", [128, 16], F32, kind="ExternalInput")
    bf2_d = nc.dram_tensor("bf2", [128, KD], F32, kind="ExternalInput")
    masks_d = nc.dram_tensor("masks", [128, 4 * 512], BF16,
                             kind="ExternalInput")
    r128_d = nc.dram_tensor("R128", [128, 128], F32R, kind="ExternalInput")
    b4_d = nc.dram_tensor("B4", [4, 128], F32R, kind="ExternalInput")
    ones_d = nc.dram_tensor("ones128", [1, 128], F32R, kind="ExternalInput")
    onescol_d = nc.dram_tensor("onescol", [128, 1], F32R,
                               kind="ExternalInput")
    iota_d = nc.dram_tensor("iota16", [16, 1], F32, kind="ExternalInput")
    eps_d = nc.dram_tensor("epsc", [1, 1], F32, kind="ExternalInput")
    outT_d = nc.dram_tensor("outT", [D, TPC], F32, kind="ExternalOutput")

    with tile.TileContext(nc) as tc:
        with tc.tile_pool(name="glob", bufs=1) as glob, \
             tc.tile_pool(name="dram", bufs=1, space="DRAM") as dram:
            # exchange slabs
            slab_in = dram.tile([BLK * H, TPC], BF16, tag="slab_in")
            slab_out = dram.tile([BLK * H, TPC], BF16, tag="slab_out")
            slab2_in = dram.tile([D, TPC], BF16, tag="slab2_in")
            slab2_out = dram.tile([D, TPC], BF16, tag="slab2_out")

            # ---- persistent constants / activations ----
            ones_sb = glob.tile([1, 128], F32R, tag="ones")
            nc.sync.dma_start(out=ones_sb[:], in_=ones_d[:])
            onescol_sb = glob.tile([128, 1], F32R, tag="onescol")
            nc.sync.dma_start(out=onescol_sb[:], in_=onescol_d[:])
            iota_sb = glob.tile([16, 1], F32, tag="iota")
            nc.sync.dma_start(out=iota_sb[:], in_=iota_d[:])
            eps_sb = glob.tile([1, 1], F32, tag="eps")
            nc.sync.dma_start(out=eps_sb[:], in_=eps_d[:])
            xT = []
            for k in range(KD):
                t = glob.tile([128, TPC], F32, tag=f"xT{k}")
                nc.sync.dma_start(out=t[:], in_=xT_d[128 * k:128 * (k + 1), :])
                xT.append(t)

            # resident fc1 weights (bf16), prefetched from the start
            w1 = []
            for k in range(KD):
                t = glob.tile([128, 2 * DFF], BF16, tag=f"w1_{k}")
                nc.sync.dma_start(out=t[:],
                                  in_=w1_d[128 * k:128 * (k + 1), :])
                w1.append(t)
            b1a_sb = glob.tile([128, 16], F32, tag="b1a")
            nc.sync.dma_start(out=b1a_sb[:], in_=b1a_d[:])
            b1g_sb = glob.tile([128, 16], F32, tag="b1g")
            nc.sync.dma_start(out=b1g_sb[:], in_=b1g_d[:])
            bf2_sb = glob.tile([128, KD], F32, tag="bf2")
            nc.sync.dma_start(out=bf2_sb[:], in_=bf2_d[:])

            def layernorm_fold(pool, src_f32r_tiles, tag):
                """6x(128,TPC) fp32r -> (ab fp32 [128,TPC], beta bf16 [1,TPC]).

                All scratch lives in a scoped pool; only the broadcast
                alpha tile and the bf16 beta row survive in `pool`.
                """
                ctx = contextlib.ExitStack()
                ps_pool = ctx.enter_context(
                    tc.tile_pool(name=f"{tag}ps", bufs=1, space="PSUM"))
                sc = ctx.enter_context(tc.tile_pool(name=f"{tag}sc", bufs=1))
                sq_pool = ctx.enter_context(
                    tc.tile_pool(name=f"{tag}sq", bufs=2))
                sums = ps_pool.tile([1, TPC], F32, tag=f"{tag}sums")
                sumsq = ps_pool.tile([1, TPC], F32, tag=f"{tag}sumsq")
                for k in range(KD):
                    sq = sq_pool.tile([128, TPC], F32R, tag=f"{tag}sqt")
                    nc.vector.tensor_tensor(
                        out=sq[:], in0=src_f32r_tiles[k][:].bitcast(F32),
                        in1=src_f32r_tiles[k][:].bitcast(F32), op=alu.mult)
                    nc.tensor.matmul(sums[:], onescol_sb[:],
                                     src_f32r_tiles[k][:],
                                     start=(k == 0), stop=(k == KD - 1))
                    nc.tensor.matmul(sumsq[:], onescol_sb[:], sq[:],
                                     start=(k == 0), stop=(k == KD - 1))
                mean = sc.tile([1, TPC], F32, tag=f"{tag}mean")
                nc.vector.tensor_scalar(out=mean[:], in0=sums[:],
                                        scalar1=1.0 / D, scalar2=None,
                                        op0=alu.mult)
                m2 = sc.tile([1, TPC], F32, tag=f"{tag}m2")
                nc.vector.tensor_tensor(out=m2[:], in0=mean[:], in1=mean[:],
                                        op=alu.mult)
                var = sc.tile([1, TPC], F32, tag=f"{tag}var")
                nc.vector.scalar_tensor_tensor(
                    out=var[:], in0=sumsq[:], scalar=1.0 / D, in1=m2[:],
                    op0=alu.mult, op1=alu.subtract)
                std = sc.tile([1, TPC], F32, tag=f"{tag}std")
                nc.scalar.activation(out=std[:], in_=var[:], func=act.Sqrt,
                                     bias=eps_sb[:])
                alpha = sc.tile([1, TPC], F32R, tag=f"{tag}alpha")
                with nc.allow_low_precision(reason="fp32r bcast rhs"):
                    nc.vector.reciprocal(out=alpha[:], in_=std[:])
                beta_bf = pool.tile([1, TPC], BF16, tag=f"{tag}betab")
                with nc.allow_low_precision(reason="rank-1 corr term"):
                    nc.vector.scalar_tensor_tensor(
                        out=beta_bf[:], in0=mean[:], scalar=-1.0,
                        in1=alpha[:].bitcast(F32), op0=alu.mult, op1=alu.mult)
                ab = ps_pool.tile([128, TPC], F32, tag=f"{tag}ab")
                nc.tensor.matmul(ab[:], ones_sb[:], alpha[:], start=True,
                                 stop=True)
                ab_sb = pool.tile([128, TPC], F32, tag=f"{tag}absb")
                nc.vector.tensor_copy(out=ab_sb[:], in_=ab[:])
                ctx.close()
                return ab_sb, beta_bf

            # ================= PHASE 1 =================
            with contextlib.ExitStack() as p1:
                p1w = p1.enter_context(tc.tile_pool(name="p1w", bufs=1))
                p1t = p1.enter_context(tc.tile_pool(name="p1t", bufs=2))

                wqk = []
                for k in range(KD):
                    t = p1w.tile([128, 1536], BF16, tag=f"wqk{k}")
                    nc.sync.dma_start(out=t[:],
                                      in_=wqk_d[128 * k:128 * (k + 1), :])
                    wqk.append(t)
                wv = []
                for k in range(KD):
                    t = p1w.tile([128, D], BF16, tag=f"wv{k}")
                    nc.sync.dma_start(out=t[:],
                                      in_=wv_d[128 * k:128 * (k + 1), :])
                    wv.append(t)
                teq_sb = p1w.tile([NTYPE, 1536], BF16, tag="teq")
                nc.sync.dma_start(out=teq_sb[:], in_=teq_d[:])
                tek_sb = p1w.tile([NTYPE, 1536], BF16, tag="tek")
                nc.sync.dma_start(out=tek_sb[:], in_=tek_d[:])
                csqk_sb = p1w.tile([1, 1536], BF16, tag="csqk")
                nc.sync.dma_start(out=csqk_sb[:], in_=csqk_d[:])
                csv_sb = p1w.tile([2, D], BF16, tag="csv")
                nc.sync.dma_start(out=csv_sb[:], in_=csv_d[:])
                r128_sb = p1w.tile([128, 128], F32R, tag="r128")
                nc.sync.dma_start(out=r128_sb[:], in_=r128_d[:])
                b4_sb = p1w.tile([4, 128], F32R, tag="b4")
                nc.sync.dma_start(out=b4_sb[:], in_=b4_d[:])
                invf_sb = p1w.tile([128, 1], F32, tag="invf")
                nc.sync.dma_start(out=invf_sb[:], in_=invf_d[:])
                pos4_sb = p1w.tile([4, TPC], F32R, tag="pos4")
                nc.sync.dma_start(out=pos4_sb[:], in_=pos4_d[:])
                qt_sb = p1w.tile([1, TPC], F32R, tag="qt")
                nc.sync.dma_start(out=qt_sb[:], in_=qtype_d[:])
                kt_sb = p1w.tile([1, TPC], F32R, tag="kt")
                nc.sync.dma_start(out=kt_sb[:], in_=ktype_d[:])

                # LN1 stats on fp32r copy of x (scratch freed after xs)
                xs = []
                with tc.tile_pool(name="p1xr", bufs=1) as p1xr:
                    xr = []
                    for k in range(KD):
                        t = p1xr.tile([128, TPC], F32R, tag=f"xr{k}")
                        nc.vector.tensor_copy(out=t[:], in_=xT[k][:])
                        xr.append(t)
                    ab1, be1b = layernorm_fold(p1w, xr, "l1")
                    for k in range(KD):
                        t = p1w.tile([128, TPC], BF16, tag=f"xs{k}")
                        with nc.allow_low_precision(reason="bf16 acts"):
                            nc.vector.tensor_tensor(out=t[:],
                                                    in0=xr[k][:].bitcast(F32),
                                                    in1=ab1[:], op=alu.mult)
                        xs.append(t)

                # beta/ones staging rows for the v correction (K=2 lhsT)
                bv2 = p1w.tile([2, TPC], BF16, tag="bv2")
                nc.vector.memset(bv2[:], 1.0)
                nc.vector.tensor_copy(out=bv2[0:1, :], in_=be1b[:])

                # one-hot type codes + cos/sin tiles; psum scratch scoped
                s_t = p1w.tile([128, TPC], F32, tag="sin")
                c_t = p1w.tile([128, TPC], F32, tag="cos")
                oh_q = p1w.tile([16, TPC], BF16, tag="qoh")
                oh_k = p1w.tile([16, TPC], BF16, tag="koh")
                with tc.tile_pool(name="p1misc", bufs=1, space="PSUM") \
                        as p1misc, \
                        tc.tile_pool(name="p1trig", bufs=1) as trig:
                    for row_sb, oh in [(qt_sb, oh_q), (kt_sb, oh_k)]:
                        bc = p1misc.tile([16, TPC], F32, tag="ohbc")
                        nc.tensor.matmul(bc[:], ones_sb[:, 0:16], row_sb[:],
                                         start=True, stop=True)
                        nc.vector.tensor_scalar(out=oh[:], in0=bc[:],
                                                scalar1=iota_sb[:],
                                                scalar2=None,
                                                op0=alu.is_equal)

                    pm = p1misc.tile([128, TPC], F32, tag="pm")
                    nc.tensor.matmul(pm[:], b4_sb[:], pos4_sb[:], start=True,
                                     stop=True)
                    f_t = trig.tile([128, TPC], F32, tag="f")
                    nc.vector.tensor_scalar(out=f_t[:], in0=pm[:],
                                            scalar1=invf_sb[:], scalar2=None,
                                            op0=alu.mult)
                    nt = trig.tile([128, TPC], F32, tag="nt")
                    nc.vector.tensor_scalar(out=nt[:], in0=f_t[:],
                                            scalar1=float(1.0 / (2 * np.pi)),
                                            scalar2=None, op0=alu.mult)
                    n_i = trig.tile([128, TPC], I32, tag="ni")
                    nc.vector.tensor_copy(out=n_i[:], in_=nt[:])
                    n_f = trig.tile([128, TPC], F32, tag="nt")
                    nc.vector.tensor_copy(out=n_f[:], in_=n_i[:])
                    fr = trig.tile([128, TPC], F32, tag="fr")
                    nc.vector.scalar_tensor_tensor(out=fr[:], in0=n_f[:],
                                                   scalar=-C1, in1=f_t[:],
                                                   op0=alu.mult, op1=alu.add)
                    nc.vector.scalar_tensor_tensor(out=fr[:], in0=n_f[:],
                                                   scalar=-C2, in1=fr[:],
                                                   op0=alu.mult, op1=alu.add)
                    nc.vector.scalar_tensor_tensor(out=fr[:], in0=n_f[:],
                                                   scalar=-C3, in1=fr[:],
                                                   op0=alu.mult, op1=alu.add)
                    nc.scalar.activation(out=s_t[:], in_=fr[:], func=act.Sin)
                    af = trig.tile([128, TPC], F32, tag="f")
                    nc.scalar.activation(out=af[:], in_=fr[:], func=act.Abs)
                    ca = trig.tile([128, TPC], F32, tag="fr")
                    nc.vector.tensor_scalar(out=ca[:], in0=af[:],
                                            scalar1=-1.0, scalar2=HALF_PI,
                                            op0=alu.mult, op1=alu.add)
                    nc.scalar.activation(out=c_t[:], in_=ca[:], func=act.Sin)
                # fold score scale 1/sqrt(HD) into q: scale c,s rows 0:64
                nc.vector.tensor_scalar(out=c_t[0:64, :], in0=c_t[0:64, :],
                                        scalar1=float(SCALE), scalar2=None,
                                        op0=alu.mult)
                nc.vector.tensor_scalar(out=s_t[0:64, :], in0=s_t[0:64, :],
                                        scalar1=float(SCALE), scalar2=None,
                                        op0=alu.mult)

                p1qk = p1.enter_context(
                    tc.tile_pool(name="p1qk", bufs=3, space="PSUM"))
                p1v = p1.enter_context(
                    tc.tile_pool(name="p1v", bufs=2, space="PSUM"))
                p1rot = p1.enter_context(
                    tc.tile_pool(name="p1rot", bufs=2, space="PSUM"))

                # qk slices: matmuls + rank-1 LN corr + type emb, then
                # rope / tails -> slab.  Tails first (trig not needed).
                for s in [8, 9, 10, 11] + list(range(8)):
                    qk_ps = p1qk.tile([128, TPC], F32, tag="qkps")
                    for k in range(KD):
                        nc.tensor.matmul(qk_ps[:],
                                         wqk[k][:, 128 * s:128 * (s + 1)],
                                         xs[k][:], start=(k == 0), stop=False)
                    nc.tensor.matmul(qk_ps[:],
                                     csqk_sb[:, 128 * s:128 * (s + 1)],
                                     be1b[:], start=False, stop=False)
                    nc.tensor.matmul(qk_ps[:],
                                     teq_sb[:, 128 * s:128 * (s + 1)],
                                     oh_q[:], start=False, stop=False)
                    nc.tensor.matmul(qk_ps[:],
                                     tek_sb[:, 128 * s:128 * (s + 1)],
                                     oh_k[:], start=False, stop=True)
                    if s < 8:
                        # rope: q_h 0:64 | k_h 0:64
                        rsb = p1t.tile([128, TPC], F32R, tag="rsb")
                        nc.vector.tensor_copy(out=rsb[:], in_=qk_ps[:])
                        rot = p1rot.tile([128, TPC], F32, tag="rot")
                        nc.tensor.matmul(rot[:], r128_sb[:], rsb[:],
                                         start=True, stop=True)
                        t1 = p1t.tile([128, TPC], F32, tag="rt1")
                        nc.vector.tensor_tensor(out=t1[:],
                                                in0=rsb[:].bitcast(F32),
                                                in1=c_t[:], op=alu.mult)
                        t2 = p1t.tile([128, TPC], F32, tag="rt2")
                        nc.vector.tensor_tensor(out=t2[:], in0=rot[:],
                                                in1=s_t[:], op=alu.mult)
                        qkr = p1t.tile([128, TPC], BF16, tag="qkr")
                        with nc.allow_low_precision(reason="bf16 payload"):
                            nc.vector.tensor_tensor(out=qkr[:], in0=t1[:],
                                                    in1=t2[:], op=alu.add)
                        h = s
                        nc.scalar.dma_start(
                            out=slab_in[BLK * h + 0:BLK * h + 64, :],
                            in_=qkr[0:64, :])
                        nc.scalar.dma_start(
                            out=slab_in[BLK * h + 96:BLK * h + 160, :],
                            in_=qkr[64:128, :])
                    else:
                        # tails: s=8,9 q tails h0..3/h4..7 (scaled), 10,11 k
                        tl = p1t.tile([128, TPC], BF16, tag="tail")
                        sc = float(SCALE) if s < 10 else 1.0
                        with nc.allow_low_precision(reason="bf16 payload"):
                            nc.vector.tensor_scalar(out=tl[:], in0=qk_ps[:],
                                                    scalar1=sc, scalar2=None,
                                                    op0=alu.mult)
                        base = 64 if s < 10 else 160
                        for j in range(4):
                            h = 4 * (s % 2) + j
                            nc.scalar.dma_start(
                                out=slab_in[BLK * h + base:BLK * h + base + 32, :],
                                in_=tl[32 * j:32 * (j + 1), :])

                # v (token-major): 4 tok-slices x 2 halves of 384 cols
                for ts_ in range(4):
                    for hf in range(2):
                        v_ps = p1v.tile([128, 384], F32, tag="vps")
                        for k in range(KD):
                            nc.tensor.matmul(
                                v_ps[:],
                                xs[k][:, 128 * ts_:128 * (ts_ + 1)],
                                wv[k][:, 384 * hf:384 * (hf + 1)],
                                start=(k == 0), stop=False)
                        nc.tensor.matmul(
                            v_ps[:], bv2[:, 128 * ts_:128 * (ts_ + 1)],
                            csv_sb[:, 384 * hf:384 * (hf + 1)],
                            start=False, stop=True)
                        v_sb1 = p1t.tile([128, 384], BF16, tag="vsb1")
                        with nc.allow_low_precision(reason="bf16 payload"):
                            nc.vector.tensor_copy(out=v_sb1[:], in_=v_ps[:])
                        # one 3D DMA: 128 tok x 4 heads x 96 feat
                        dst = bass.AP(
                            tensor=slab_in[:].tensor,
                            offset=(BLK * 4 * hf + 192) * TPC + 128 * ts_ * 96,
                            ap=[[96, 128], [BLK * TPC, 4], [1, 96]])
                        src = bass.AP(
                            tensor=v_sb1[:].tensor,
                            offset=v_sb1[:].offset,
                            ap=[[384, 128], [96, 4], [1, 96]])
                        nc.scalar.dma_start(out=dst, in_=src)

            nc.gpsimd.collective_compute(
                "AllToAll", mybir.AluOpType.bypass,
                replica_groups=[list(range(N_CORES))],
                ins=[slab_in[:].opt()],
                outs=[slab_out[:].opt()])

            # ================= PHASE 2 =================
            with contextlib.ExitStack() as p2:
                p2w = p2.enter_context(tc.tile_pool(name="p2w", bufs=1))
                p2t = p2.enter_context(tc.tile_pool(name="p2t", bufs=3))
                p2ps = p2.enter_context(
                    tc.tile_pool(name="p2ps", bufs=2, space="PSUM"))
                p2o = p2.enter_context(
                    tc.tile_pool(name="p2o", bufs=2, space="PSUM"))
                p2rb = p2.enter_context(
                    tc.tile_pool(name="p2rb", bufs=2, space="PSUM"))

                masks_sb = p2w.tile([128, 4 * 512], BF16, tag="masks")
                nc.sync.dma_start(out=masks_sb[:], in_=masks_d[:])

                for bb_ in range(2):
                    qT = p2w.tile([96, 2048], BF16, tag=f"qT{bb_}")
                    kT = p2w.tile([96, 2048], BF16, tag=f"kT{bb_}")
                    v_sb = p2w.tile([128, 16, 97], BF16, tag=f"v{bb_}")
                    nc.vector.memset(v_sb[:, :, 96:97], 1.0)
                    for u in range(4):
                        blk = BLK * (4 * bb_ + u)
                        nc.sync.dma_start(
                            out=qT[:, 512 * u:512 * (u + 1)],
                            in_=slab_out[blk + 0:blk + 96, :])
                        nc.sync.dma_start(
                            out=kT[:, 512 * u:512 * (u + 1)],
                            in_=slab_out[blk + 96:blk + 192, :])
                        # one 3D DMA for v: 128 tok x 4 tok-chunks x 96 feat
                        src = bass.AP(
                            tensor=slab_out[:].tensor,
                            offset=(blk + 192) * TPC,
                            ap=[[96, 128], [128 * 96, 4], [1, 96]])
                        dst = bass.AP(
                            tensor=v_sb[:].tensor,
                            offset=v_sb[:].offset + 97 * 4 * u,
                            ap=[[16 * 97, 128], [97, 4], [1, 96]])
                        nc.sync.dma_start(out=dst, in_=src)

                    for Q in range(NSUP):
                        o_ps = p2o.tile([97, 512], F32, tag="ops", name="ops")
                        npair = 2 * Q + 2
                        for pr in range(npair):
                            s_ps = p2ps.tile([128, 1024], F32, tag="sps",
                                             name="sps")
                            for half in range(2):
                                kt = 2 * pr + half
                                nc.tensor.matmul(
                                    s_ps[:, 512 * half:512 * (half + 1)],
                                    kT[:, 128 * kt:128 * (kt + 1)],
                                    qT[:, 512 * Q:512 * (Q + 1)],
                                    start=True, stop=True)
                            e_sb = p2t.tile([128, 1024], BF16, tag="esb",
                                            name="esb")
                            with nc.allow_low_precision(reason="bf16 probs"):
                                nc.scalar.activation(out=e_sb[:], in_=s_ps[:],
                                                     func=act.Exp)
                            for half in range(2):
                                kt = 2 * pr + half
                                dj = kt - 4 * Q
                                if dj >= 0:
                                    eh = e_sb[:, 512 * half:512 * (half + 1)]
                                    nc.vector.tensor_tensor(
                                        out=eh, in0=eh,
                                        in1=masks_sb[:, 512 * dj:512 * (dj + 1)],
                                        op=alu.mult)
                            for half in range(2):
                                kt = 2 * pr + half
                                nc.tensor.matmul(
                                    o_ps[:], v_sb[:, kt, :],
                                    e_sb[:, 512 * half:512 * (half + 1)],
                                    start=(kt == 0),
                                    stop=(kt == 4 * Q + 3))
                        # per-unit softmax tail straight out of PSUM
                        j = 4 * bb_ + Q
                        rec = p2t.tile([1, 512], F32R, tag="rec", name="rec")
                        with nc.allow_low_precision(reason="softmax recip"):
                            nc.vector.reciprocal(out=rec[:],
                                                 in_=o_ps[96:97, :])
                        rb = p2rb.tile([96, 512], F32, tag="rb", name="rb")
                        nc.tensor.matmul(rb[:], ones_sb[:, 0:96], rec[:],
                                         start=True, stop=True)
                        rb_sb = p2t.tile([96, 512], F32, tag="rbsb",
                                         name="rbsb")
                        nc.vector.tensor_copy(out=rb_sb[:], in_=rb[:])
                        onrm = p2t.tile([96, 512], BF16, tag="onrm",
                                        name="onrm")
                        with nc.allow_low_precision(reason="bf16 payload"):
                            nc.vector.tensor_tensor(out=onrm[:],
                                                    in0=o_ps[0:96, :],
                                                    in1=rb_sb[:], op=alu.mult)
                        nc.scalar.dma_start(
                            out=slab2_in[96 * j:96 * (j + 1), :], in_=onrm[:])

            nc.gpsimd.collective_compute(
                "AllToAll", mybir.AluOpType.bypass,
                replica_groups=[list(range(N_CORES))],
                ins=[slab2_in[:].opt()], outs=[slab2_out[:].opt()])

            # ================= PHASE 3 =================
            with contextlib.ExitStack() as p3:
                p3w = p3.enter_context(tc.tile_pool(name="p3w", bufs=1))
                p3t = p3.enter_context(tc.tile_pool(name="p3t", bufs=2))
                p3ps = p3.enter_context(
                    tc.tile_pool(name="p3ps", bufs=2, space="PSUM"))

                cs1_sb = p3w.tile([1, 2 * DFF], BF16, tag="cs1")
                nc.sync.dma_start(out=cs1_sb[:], in_=cs1_d[:])
                w2p = p3.enter_context(tc.tile_pool(name="p3w2", bufs=3))

                x2 = []
                x2r = []
                for k in range(KD):
                    o_sb = p3t.tile([128, TPC], BF16, tag="osb")
                    nc.sync.dma_start(out=o_sb[:],
                                      in_=slab2_out[128 * k:128 * (k + 1), :])
                    t = p3w.tile([128, TPC], F32, tag=f"x2_{k}")
                    nc.vector.tensor_tensor(out=t[:], in0=o_sb[:],
                                            in1=xT[k][:], op=alu.add)
                    x2.append(t)
                    tr = p3w.tile([128, TPC], F32R, tag=f"x2r{k}")
                    nc.vector.tensor_copy(out=tr[:], in_=t[:])
                    x2r.append(tr)

                ab2, be2b = layernorm_fold(p3w, x2r, "l2")
                xs2 = []
                for k in range(KD):
                    t = p3w.tile([128, TPC], BF16, tag=f"xs2_{k}")
                    with nc.allow_low_precision(reason="bf16 activations"):
                        nc.vector.tensor_tensor(out=t[:],
                                                in0=x2r[k][:].bitcast(F32),
                                                in1=ab2[:], op=alu.mult)
                    xs2.append(t)

                # fused fc1 + silu + fc2 stream over 16 dff chunks
                with tc.tile_pool(name="p3f", bufs=1, space="PSUM") as p3f:
                    ff_ps = [p3f.tile([128, TPC], F32, tag=f"ff{d}",
                                      name=f"ff{d}")
                             for d in range(KD)]
                    for i in range(16):
                        a_ps = p3ps.tile([128, TPC], F32, tag="hps",
                                         name="aps")
                        for k in range(KD):
                            nc.tensor.matmul(
                                a_ps[:],
                                w1[k][:, 128 * i:128 * (i + 1)],
                                xs2[k][:], start=(k == 0), stop=False)
                        nc.tensor.matmul(
                            a_ps[:], cs1_sb[:, 128 * i:128 * (i + 1)],
                            be2b[:], start=False, stop=True)
                        g_ps = p3ps.tile([128, TPC], F32, tag="hps",
                                         name="gps")
                        for k in range(KD):
                            nc.tensor.matmul(
                                g_ps[:],
                                w1[k][:, 2048 + 128 * i:2048 + 128 * (i + 1)],
                                xs2[k][:], start=(k == 0), stop=False)
                        nc.tensor.matmul(
                            g_ps[:],
                            cs1_sb[:, 2048 + 128 * i:2048 + 128 * (i + 1)],
                            be2b[:], start=False, stop=True)
                        a_sb = p3t.tile([128, TPC], F32, tag="asb")
                        nc.vector.tensor_scalar(
                            out=a_sb[:], in0=a_ps[:],
                            scalar1=b1a_sb[:, i:i + 1],
                            scalar2=None, op0=alu.add)
                        sil = p3t.tile([128, TPC], F32, tag="sil")
                        nc.scalar.activation(
                            out=sil[:], in_=g_ps[:], func=act.Silu,
                            bias=b1g_sb[:, i:i + 1])
                        sw = p3t.tile([128, TPC], BF16, tag="sw")
                        with nc.allow_low_precision(reason="bf16 ffn acts"):
                            nc.vector.tensor_tensor(out=sw[:], in0=sil[:],
                                                    in1=a_sb[:], op=alu.mult)
                        w2c = w2p.tile([128, D], BF16, tag="w2c")
                        nc.sync.dma_start(
                            out=w2c[:], in_=w2_d[128 * i:128 * (i + 1), :])
                        for d in range(KD):
                            nc.tensor.matmul(ff_ps[d][:],
                                             w2c[:, 128 * d:128 * (d + 1)],
                                             sw[:],
                                             start=(i == 0), stop=(i == 15))
                    for d in range(KD):
                        o = p3t.tile([128, TPC], F32, tag="oout")
                        nc.vector.scalar_tensor_tensor(
                            out=o[:], in0=ff_ps[d][:],
                            scalar=bf2_sb[:, d:d + 1], in1=x2[d][:],
                            op0=alu.add, op1=alu.add)
                        nc.sync.dma_start(
                            out=outT_d[128 * d:128 * (d + 1), :], in_=o[:])

    nc.compile()
    _prog_cache[key] = nc
    return nc


def _host_inputs(x_type, x_value, seq_order, W_attn, type_emb, g1, b1, g2, b2,
                 W_fc1, b_fc1, W_fc2, b_fc2):
    f32 = np.float32
    bf16 = mybir.dt.np(mybir.dt.bfloat16)
    x_type = np.asarray(x_type)
    seq_order = np.asarray(seq_order)
    x_value = np.asarray(x_value, dtype=f32)
    W_attn = np.asarray(W_attn, dtype=f32)
    type_emb = np.asarray(type_emb, dtype=f32)
    W_fc1 = np.asarray(W_fc1, dtype=f32)
    W_fc2 = np.asarray(W_fc2, dtype=f32)
    g1 = np.asarray(g1, f32); b1 = np.asarray(b1, f32)
    g2 = np.asarray(g2, f32); b2 = np.asarray(b2, f32)
    b_fc1 = np.asarray(b_fc1, f32); b_fc2 = np.asarray(b_fc2, f32)

    # ---- qk weights: permute, fold g1; fold b1 bias into type emb ----
    w_perm = W_attn[:, :1536][:, QK_PERM]
    wqk_bf = (w_perm * g1[:, None]).astype(bf16)
    csqk = wqk_bf.astype(f32).sum(axis=0, keepdims=True).astype(bf16)
    bias_qk = (b1 @ w_perm).astype(f32)          # (1536,)
    te_full = type_emb[:, QK_PERM]               # (16, 1536)
    q_origin = QK_PERM < 768
    te_q = np.where(q_origin[None, :], te_full + bias_qk[None, :], 0.0)
    te_k = np.where(~q_origin[None, :], te_full + bias_qk[None, :], 0.0)

    # ---- v weights: fold g1; beta/bias correction rows ----
    wv_bf = (W_attn[:, 1536:] * g1[:, None]).astype(bf16)
    csv = np.stack([
        wv_bf.astype(f32).sum(axis=0),           # colsum row (x beta)
        (b1 @ W_attn[:, 1536:]).astype(f32),     # bias row (x ones)
    ]).astype(bf16)

    # ---- ffn weights: fold g2; fold b2 into fc1 bias ----
    w1_bf = (W_fc1 * g2[:, None]).astype(bf16)
    cs1 = w1_bf.astype(f32).sum(axis=0, keepdims=True).astype(bf16)
    bias_fc1 = (b_fc1 + b2 @ W_fc1).astype(f32)  # (4096,)
    w2_bf = W_fc2.astype(bf16)

    invf16 = (1.0 / THETA ** (np.arange(0, DR, 2, dtype=f32) / DR)).astype(f32)
    invf_col = invf16[(np.arange(128) % 32) // 2].reshape(128, 1)

    # masks: block (128k x 512q), mask[kk, qq] = 1 if qq >= kk + 128*dj
    kk = np.arange(128)[:, None]
    qq = np.arange(512)[None, :]
    masks = np.concatenate(
        [(qq >= kk + 128 * dj).astype(f32) for dj in range(4)],
        axis=1).astype(bf16)

    # rot lhsT: lhsT[k, m] = P[m, k];  P[2i, 2i+1] = -1, P[2i+1, 2i] = +1
    R = np.zeros((128, 128), f32)
    for i in range(64):
        R[2 * i + 1, 2 * i] = -1.0
        R[2 * i, 2 * i + 1] = 1.0
    B4m = np.zeros((4, 128), f32)
    B4m[0, 0:32] = 1.0; B4m[1, 32:64] = 1.0
    B4m[2, 64:96] = 1.0; B4m[3, 96:128] = 1.0

    common = {
        "Wqk": wqk_bf, "Wv": wv_bf,
        "te_q": te_q.astype(bf16), "te_k": te_k.astype(bf16),
        "csqk": csqk, "csv": csv,
        "invf": invf_col,
        "W1": w1_bf, "cs1": cs1, "W2": w2_bf,
        "b1a": bias_fc1[:2048].reshape(16, 128).T.copy(),
        "b1g": bias_fc1[2048:].reshape(16, 128).T.copy(),
        "bf2": b_fc2.reshape(6, 128).T.copy(),
        "masks": masks, "R128": R, "B4": B4m,
        "ones128": np.ones((1, 128), f32),
        "onescol": np.ones((128, 1), f32),
        "iota16": np.arange(16, dtype=f32).reshape(16, 1),
        "epsc": np.full((1, 1), EPS, f32),
    }
    in_maps = []
    for c in range(N_CORES):
        b = c // 4
        t0 = 512 * (c % 4)
        m = dict(common)
        m["xT"] = np.ascontiguousarray(x_value[b, t0:t0 + TPC, :].T)
        m["qtype"] = x_type[b, t0:t0 + TPC].astype(f32).reshape(1, TPC)
        m["ktype"] = x_type[b, t0 + 1:t0 + TPC + 1].astype(f32).reshape(1, TPC)
        pos4 = np.stack([
            seq_order[0, b, t0:t0 + TPC],
            seq_order[1, b, t0:t0 + TPC],
            seq_order[0, b, t0 + 1:t0 + TPC + 1],
            seq_order[1, b, t0 + 1:t0 + TPC + 1],
        ]).astype(f32)
        m["pos4"] = pos4
        in_maps.append(m)
    return in_maps


def kernel(**inputs):
    nc = build_program()
    in_maps = _host_inputs(**inputs)
    res = run_bass_kernel_spmd(nc, in_maps, list(range(N_CORES)), trace=False)
    out = np.empty((B, T, D), np.float32)
    for c in range(N_CORES):
        b = c // 4
        t0 = 512 * (c % 4)
        out[b, t0:t0 + TPC, :] = res.results[c]["outT"].T
    return out


# revision 32
# speedup vs baseline: 1.1060x; 1.1060x over previous
"""Trainium2 Bass kernel for nn_DecoderLayer_90967407329666.

Decoder layer: LN1 -> QKV (+type emb) -> multi-axis RoPE -> causal SDPA
-> residual -> LN2 -> SwiGLU FFN -> residual.  B=2, T=2048, D=768, H=8,
DFF=2048, NTYPE=16, NAX=2 rotary axes of 32 dims each.

Sharding (8 cores):
  Phase 1 (token-parallel): core c owns 512 tokens (batch c//4, tokens
    512*(c%4)..). LN1 is folded into the projections: q/k/v come from
    matmuls on xs = x*alpha (bf16) plus a rank-1 mean-correction matmul
    (colsum(W) (x) beta) accumulated in PSUM; the b1 bias is folded into
    the type-emb tables host-side.
  One merged AllToAll moves a [2304, 512] bf16 slab (per head h: 96 rows
    q | 96 rows k | 96 rows v-token-major) so core c ends up with head c
    for all 4096 tokens.
  Phase 2 (head-parallel): causal attention per head, exp without max
    subtraction, score tiles paired into [128,1024] PSUM so one Exp
    covers two k-tiles, per-unit softmax tail straight out of PSUM.
  AllToAll #2 returns o (bf16) token-parallel.
  Phase 3: residual + folded LN2 + fused fc1+silu+fc2 stream + residual.
    W1/W2 are prefetched in bf16 at phase-1 start and stay resident.

All heavy matmuls run with bf16 operands (full PE rate); the residual
stream stays fp32; stats run in fp32r.
"""

import sys

sys.path.insert(0, "/opt/trn_rl_repo")

import numpy as np

import contextlib

import concourse.bacc as bacc
import concourse.bass as bass
import concourse.tile as tile
from concourse import mybir
from concourse.bass_utils import run_bass_kernel_spmd

# ---- problem constants (hardcoded per contest rules) ----
B, T = 2, 2048
D, H, DFF, NTYPE = 768, 8, 2048, 16
NAX = 2
HD = D // H            # 96
DR = HD // (NAX + 1)   # 32
EPS = 1e-5
THETA = 10000.0
N_CORES = 8
TPC = 512              # tokens per core
NSUP = 4               # supertiles per batch (2048/512)
KD = D // 128          # 6 contraction chunks over D
SCALE = 1.0 / np.sqrt(np.float32(HD))

F32 = mybir.dt.float32
F32R = mybir.dt.float32r
BF16 = mybir.dt.bfloat16
I32 = mybir.dt.int32

# Cody-Waite split of 2*pi (C1 has 12 mantissa bits -> n*C1 exact for n<2^11)
C1 = float(np.float32(np.floor(2 * np.pi * 2**9) / 2**9))
C2 = float(np.float32(2 * np.pi - C1))
C3 = float(np.float32(2 * np.pi - C1 - float(np.float32(2 * np.pi - C1))))
HALF_PI = float(np.pi / 2)

# qk output-feature permutation: 12 slices of 128 rows
#   slices 0..7  : [q_h dims 0:64 | k_h dims 0:64]   (rope rows)
#   slice  8, 9  : q tails (dims 64:96) of heads 0..3 / 4..7
#   slice 10,11  : k tails of heads 0..3 / 4..7
def _qk_colperm():
    cols = []
    for h in range(H):
        cols += list(range(96 * h, 96 * h + 64))          # q_h 0:64
        cols += list(range(768 + 96 * h, 768 + 96 * h + 64))  # k_h 0:64
    for h in range(H):
        cols += list(range(96 * h + 64, 96 * h + 96))     # q tails
    for h in range(H):
        cols += list(range(768 + 96 * h + 64, 768 + 96 * h + 96))  # k tails
    return np.array(cols)

QK_PERM = _qk_colperm()

_prog_cache = {}

# merged slab geometry: per head h, rows 288h+0:96 q | +96:192 k |
# +192:288 v (token-major inside)
BLK = 288


def build_program():
    key = 0
    if key in _prog_cache:
        return _prog_cache[key]
    nc = bacc.Bacc("TRN2", target_bir_lowering=False, debug=False,
                   num_devices=N_CORES)
    alu = mybir.AluOpType
    act = mybir.ActivationFunctionType

    # ---------------- DRAM I/O ----------------
    xT_d = nc.dram_tensor("xT", [D, TPC], F32, kind="ExternalInput")
    wqk_d = nc.dram_tensor("Wqk", [D, 1536], BF16, kind="ExternalInput")
    wv_d = nc.dram_tensor("Wv", [D, D], BF16, kind="ExternalInput")
    teq_d = nc.dram_tensor("te_q", [NTYPE, 1536], BF16, kind="ExternalInput")
    tek_d = nc.dram_tensor("te_k", [NTYPE, 1536], BF16, kind="ExternalInput")
    csqk_d = nc.dram_tensor("csqk", [1, 1536], BF16, kind="ExternalInput")
    csv_d = nc.dram_tensor("csv", [2, D], BF16, kind="ExternalInput")
    qtype_d = nc.dram_tensor("qtype", [1, TPC], F32R, kind="ExternalInput")
    ktype_d = nc.dram_tensor("ktype", [1, TPC], F32R, kind="ExternalInput")
    pos4_d = nc.dram_tensor("pos4", [4, TPC], F32R, kind="ExternalInput")
    invf_d = nc.dram_tensor("invf", [128, 1], F32, kind="ExternalInput")
    w1_d = nc.dram_tensor("W1", [D, 2 * DFF], BF16, kind="ExternalInput")
    cs1_d = nc.dram_tensor("cs1", [1, 2 * DFF], BF16, kind="ExternalInput")
    w2_d = nc.dram_tensor("W2", [DFF, D], BF16, kind="ExternalInput")
    b1a_d = nc.dram_tensor("b1a", [128, 16], F32, kind="ExternalInput")
    b1g_d = nc.dram_tensor("b1g", [128, 16], F32, kind="ExternalInput")
    bf2_d = nc.dram_tensor("bf2", [128, KD], F32, kind="ExternalInput")
    masks_d = nc.dram_tensor("masks", [128, 4 * 512], BF16,
                             kind="ExternalInput")
    r128_d = nc.dram_tensor("R128", [128, 128], F32R, kind="ExternalInput")
    b4_d = nc.dram_tensor("B4", [4, 128], F32R, kind="ExternalInput")
    id96_d = nc.dram_tensor("id96", [96, 96], BF16, kind="ExternalInput")
    ones_d = nc.dram_tensor("ones128", [1, 128], F32R, kind="ExternalInput")
    onescol_d = nc.dram_tensor("onescol", [128, 1], F32R,
                               kind="ExternalInput")
    iota_d = nc.dram_tensor("iota16", [16, 1], F32, kind="ExternalInput")
    eps_d = nc.dram_tensor("epsc", [1, 1], F32, kind="ExternalInput")
    outT_d = nc.dram_tensor("outT", [D, TPC], F32, kind="ExternalOutput")

    with tile.TileContext(nc) as tc:
        with tc.tile_pool(name="glob", bufs=1) as glob, \
             tc.tile_pool(name="dram", bufs=1, space="DRAM") as dram:
            # exchange slabs
            slab_in = dram.tile([BLK * H, TPC], BF16, tag="slab_in")
            slab_out = dram.tile([BLK * H, TPC], BF16, tag="slab_out")
            slab2_in = dram.tile([D, TPC], BF16, tag="slab2_in")
            slab2_out = dram.tile([D, TPC], BF16, tag="slab2_out")

            # ---- persistent constants / activations ----
            ones_sb = glob.tile([1, 128], F32R, tag="ones")
            nc.sync.dma_start(out=ones_sb[:], in_=ones_d[:])
            onescol_sb = glob.tile([128, 1], F32R, tag="onescol")
            nc.sync.dma_start(out=onescol_sb[:], in_=onescol_d[:])
            iota_sb = glob.tile([16, 1], F32, tag="iota")
            nc.sync.dma_start(out=iota_sb[:], in_=iota_d[:])
            eps_sb = glob.tile([1, 1], F32, tag="eps")
            nc.sync.dma_start(out=eps_sb[:], in_=eps_d[:])
            xT = []
            for k in range(KD):
                t = glob.tile([128, TPC], F32, tag=f"xT{k}")
                nc.sync.dma_start(out=t[:], in_=xT_d[128 * k:128 * (k + 1), :])
                xT.append(t)

            # resident ffn weights (bf16); DMAs issued after the
            # phase-1-critical loads so they don't delay Wqk/Wv
            w1 = [glob.tile([128, 2 * DFF], BF16, tag=f"w1_{k}",
                            name=f"w1_{k}")
                  for k in range(KD)]
            w2 = [glob.tile([128, D], BF16, tag=f"w2_{k2}",
                            name=f"w2_{k2}")
                  for k2 in range(16)]
            b1a_sb = glob.tile([128, 16], F32, tag="b1a")
            nc.sync.dma_start(out=b1a_sb[:], in_=b1a_d[:])
            b1g_sb = glob.tile([128, 16], F32, tag="b1g")
            nc.sync.dma_start(out=b1g_sb[:], in_=b1g_d[:])
            bf2_sb = glob.tile([128, KD], F32, tag="bf2")
            nc.sync.dma_start(out=bf2_sb[:], in_=bf2_d[:])

            def layernorm_fold(pool, src_f32r_tiles, tag):
                """6x(128,TPC) fp32r -> (ab fp32 [128,TPC], beta bf16 [1,TPC]).

                All scratch lives in a scoped pool; only the broadcast
                alpha tile and the bf16 beta row survive in `pool`.
                """
                ctx = contextlib.ExitStack()
                ps_pool = ctx.enter_context(
                    tc.tile_pool(name=f"{tag}ps", bufs=1, space="PSUM"))
                sc = ctx.enter_context(tc.tile_pool(name=f"{tag}sc", bufs=1))
                sq_pool = ctx.enter_context(
                    tc.tile_pool(name=f"{tag}sq", bufs=2))
                sums = ps_pool.tile([1, TPC], F32, tag=f"{tag}sums")
                sumsq = ps_pool.tile([1, TPC], F32, tag=f"{tag}sumsq")
                for k in range(KD):
                    sq = sq_pool.tile([128, TPC], F32R, tag=f"{tag}sqt")
                    nc.vector.tensor_tensor(
                        out=sq[:], in0=src_f32r_tiles[k][:].bitcast(F32),
                        in1=src_f32r_tiles[k][:].bitcast(F32), op=alu.mult)
                    nc.tensor.matmul(sums[:], onescol_sb[:],
                                     src_f32r_tiles[k][:],
                                     start=(k == 0), stop=(k == KD - 1))
                    nc.tensor.matmul(sumsq[:], onescol_sb[:], sq[:],
                                     start=(k == 0), stop=(k == KD - 1))
                mean = sc.tile([1, TPC], F32, tag=f"{tag}mean")
                nc.vector.tensor_scalar(out=mean[:], in0=sums[:],
                                        scalar1=1.0 / D, scalar2=None,
                                        op0=alu.mult)
                m2 = sc.tile([1, TPC], F32, tag=f"{tag}m2")
                nc.vector.tensor_tensor(out=m2[:], in0=mean[:], in1=mean[:],
                                        op=alu.mult)
                var = sc.tile([1, TPC], F32, tag=f"{tag}var")
                nc.vector.scalar_tensor_tensor(
                    out=var[:], in0=sumsq[:], scalar=1.0 / D, in1=m2[:],
                    op0=alu.mult, op1=alu.subtract)
                std = sc.tile([1, TPC], F32, tag=f"{tag}std")
                nc.scalar.activation(out=std[:], in_=var[:], func=act.Sqrt,
                                     bias=eps_sb[:])
                alpha = sc.tile([1, TPC], F32R, tag=f"{tag}alpha")
                with nc.allow_low_precision(reason="fp32r bcast rhs"):
                    nc.vector.reciprocal(out=alpha[:], in_=std[:])
                beta_bf = pool.tile([1, TPC], BF16, tag=f"{tag}betab")
                with nc.allow_low_precision(reason="rank-1 corr term"):
                    nc.vector.scalar_tensor_tensor(
                        out=beta_bf[:], in0=mean[:], scalar=-1.0,
                        in1=alpha[:].bitcast(F32), op0=alu.mult, op1=alu.mult)
                ab = ps_pool.tile([128, TPC], F32, tag=f"{tag}ab")
                nc.tensor.matmul(ab[:], ones_sb[:], alpha[:], start=True,
                                 stop=True)
                ab_sb = pool.tile([128, TPC], F32, tag=f"{tag}absb")
                nc.vector.tensor_copy(out=ab_sb[:], in_=ab[:])
                ctx.close()
                return ab_sb, beta_bf

            # ================= PHASE 1 =================
            with contextlib.ExitStack() as p1:
                p1w = p1.enter_context(tc.tile_pool(name="p1w", bufs=1))
                p1t = p1.enter_context(tc.tile_pool(name="p1t", bufs=2))

                wqk = []
                for k in range(KD):
                    t = p1w.tile([128, 1536], BF16, tag=f"wqk{k}")
                    nc.sync.dma_start(out=t[:],
                                      in_=wqk_d[128 * k:128 * (k + 1), :])
                    wqk.append(t)
                wv = []
                for k in range(KD):
                    t = p1w.tile([128, D], BF16, tag=f"wv{k}")
                    nc.sync.dma_start(out=t[:],
                                      in_=wv_d[128 * k:128 * (k + 1), :])
                    wv.append(t)
                teq_sb = p1w.tile([NTYPE, 1536], BF16, tag="teq")
                nc.sync.dma_start(out=teq_sb[:], in_=teq_d[:])
                tek_sb = p1w.tile([NTYPE, 1536], BF16, tag="tek")
                nc.sync.dma_start(out=tek_sb[:], in_=tek_d[:])
                csqk_sb = p1w.tile([1, 1536], BF16, tag="csqk")
                nc.sync.dma_start(out=csqk_sb[:], in_=csqk_d[:])
                csv_sb = p1w.tile([2, D], BF16, tag="csv")
                nc.sync.dma_start(out=csv_sb[:], in_=csv_d[:])
                r128_sb = p1w.tile([128, 128], F32R, tag="r128")
                nc.sync.dma_start(out=r128_sb[:], in_=r128_d[:])
                b4_sb = p1w.tile([4, 128], F32R, tag="b4")
                nc.sync.dma_start(out=b4_sb[:], in_=b4_d[:])
                invf_sb = p1w.tile([128, 1], F32, tag="invf")
                nc.sync.dma_start(out=invf_sb[:], in_=invf_d[:])
                pos4_sb = p1w.tile([4, TPC], F32R, tag="pos4")
                nc.sync.dma_start(out=pos4_sb[:], in_=pos4_d[:])
                qt_sb = p1w.tile([1, TPC], F32R, tag="qt")
                nc.sync.dma_start(out=qt_sb[:], in_=qtype_d[:])
                kt_sb = p1w.tile([1, TPC], F32R, tag="kt")
                nc.sync.dma_start(out=kt_sb[:], in_=ktype_d[:])
                # ffn weight prefetch (streams under phase 1 + collective)
                for k in range(KD):
                    nc.sync.dma_start(out=w1[k][:],
                                      in_=w1_d[128 * k:128 * (k + 1), :])
                for k2 in range(16):
                    nc.sync.dma_start(out=w2[k2][:],
                                      in_=w2_d[128 * k2:128 * (k2 + 1), :])

                # LN1 stats on fp32r copy of x (scratch freed after xs)
                xs = []
                with tc.tile_pool(name="p1xr", bufs=1) as p1xr:
                    xr = []
                    for k in range(KD):
                        t = p1xr.tile([128, TPC], F32R, tag=f"xr{k}")
                        nc.vector.tensor_copy(out=t[:], in_=xT[k][:])
                        xr.append(t)
                    ab1, be1b = layernorm_fold(p1w, xr, "l1")
                    for k in range(KD):
                        t = p1w.tile([128, TPC], BF16, tag=f"xs{k}")
                        with nc.allow_low_precision(reason="bf16 acts"):
                            nc.vector.tensor_tensor(out=t[:],
                                                    in0=xr[k][:].bitcast(F32),
                                                    in1=ab1[:], op=alu.mult)
                        xs.append(t)

                # beta/ones staging rows for the v correction (K=2 lhsT)
                bv2 = p1w.tile([2, TPC], BF16, tag="bv2")
                nc.vector.memset(bv2[:], 1.0)
                nc.vector.tensor_copy(out=bv2[0:1, :], in_=be1b[:])

                # one-hot type codes + cos/sin tiles; psum scratch scoped
                s_t = p1w.tile([128, TPC], F32, tag="sin")
                c_t = p1w.tile([128, TPC], F32, tag="cos")
                oh_q = p1w.tile([16, TPC], BF16, tag="qoh")
                oh_k = p1w.tile([16, TPC], BF16, tag="koh")
                with tc.tile_pool(name="p1misc", bufs=1, space="PSUM") \
                        as p1misc, \
                        tc.tile_pool(name="p1trig", bufs=1) as trig:
                    for row_sb, oh in [(qt_sb, oh_q), (kt_sb, oh_k)]:
                        bc = p1misc.tile([16, TPC], F32, tag="ohbc")
                        nc.tensor.matmul(bc[:], ones_sb[:, 0:16], row_sb[:],
                                         start=True, stop=True)
                        nc.vector.tensor_scalar(out=oh[:], in0=bc[:],
                                                scalar1=iota_sb[:],
                                                scalar2=None,
                                                op0=alu.is_equal)

                    pm = p1misc.tile([128, TPC], F32, tag="pm")
                    nc.tensor.matmul(pm[:], b4_sb[:], pos4_sb[:], start=True,
                                     stop=True)
                    f_t = trig.tile([128, TPC], F32, tag="f")
                    nc.vector.tensor_scalar(out=f_t[:], in0=pm[:],
                                            scalar1=invf_sb[:], scalar2=None,
                                            op0=alu.mult)
                    nt = trig.tile([128, TPC], F32, tag="nt")
                    nc.vector.tensor_scalar(out=nt[:], in0=f_t[:],
                                            scalar1=float(1.0 / (2 * np.pi)),
                                            scalar2=None, op0=alu.mult)
                    n_i = trig.tile([128, TPC], I32, tag="ni")
                    nc.vector.tensor_copy(out=n_i[:], in_=nt[:])
                    n_f = trig.tile([128, TPC], F32, tag="nt")
                    nc.vector.tensor_copy(out=n_f[:], in_=n_i[:])
                    fr = trig.tile([128, TPC], F32, tag="fr")
                    nc.vector.scalar_tensor_tensor(out=fr[:], in0=n_f[:],
                                                   scalar=-C1, in1=f_t[:],
                                                   op0=alu.mult, op1=alu.add)
                    nc.vector.scalar_tensor_tensor(out=fr[:], in0=n_f[:],
                                                   scalar=-C2, in1=fr[:],
                                                   op0=alu.mult, op1=alu.add)
                    nc.vector.scalar_tensor_tensor(out=fr[:], in0=n_f[:],
                                                   scalar=-C3, in1=fr[:],
                                                   op0=alu.mult, op1=alu.add)
                    nc.scalar.activation(out=s_t[:], in_=fr[:], func=act.Sin)
                    af = trig.tile([128, TPC], F32, tag="f")
                    nc.scalar.activation(out=af[:], in_=fr[:], func=act.Abs)
                    ca = trig.tile([128, TPC], F32, tag="fr")
                    nc.vector.tensor_scalar(out=ca[:], in0=af[:],
                                            scalar1=-1.0, scalar2=HALF_PI,
                                            op0=alu.mult, op1=alu.add)
                    nc.scalar.activation(out=c_t[:], in_=ca[:], func=act.Sin)
                # fold score scale 1/sqrt(HD) into q: scale c,s rows 0:64
                nc.vector.tensor_scalar(out=c_t[0:64, :], in0=c_t[0:64, :],
                                        scalar1=float(SCALE), scalar2=None,
                                        op0=alu.mult)
                nc.vector.tensor_scalar(out=s_t[0:64, :], in0=s_t[0:64, :],
                                        scalar1=float(SCALE), scalar2=None,
                                        op0=alu.mult)

                p1qk = p1.enter_context(
                    tc.tile_pool(name="p1qk", bufs=3, space="PSUM"))
                p1v = p1.enter_context(
                    tc.tile_pool(name="p1v", bufs=2, space="PSUM"))
                p1rot = p1.enter_context(
                    tc.tile_pool(name="p1rot", bufs=2, space="PSUM"))

                # v slices (feature-major, like q/k): slice sv covers
                # v-features 128sv..128sv+128 -> heads via row split
                def v_slice(sv):
                    v_ps = p1v.tile([128, TPC], F32, tag="vps")
                    for k in range(KD):
                        nc.tensor.matmul(v_ps[:],
                                         wv[k][:, 128 * sv:128 * (sv + 1)],
                                         xs[k][:], start=(k == 0), stop=False)
                    nc.tensor.matmul(
                        v_ps[:], csv_sb[:, 128 * sv:128 * (sv + 1)],
                        bv2[:, :], start=False, stop=True)
                    v_sb1 = p1t.tile([128, TPC], BF16, tag="vsb1")
                    with nc.allow_low_precision(reason="bf16 payload"):
                        nc.vector.tensor_copy(out=v_sb1[:], in_=v_ps[:])
                    # rows map to heads: feature f = 128*sv + r, head f//96
                    r = 0
                    while r < 128:
                        f = 128 * sv + r
                        h = f // 96
                        nrow = min(128 - r, 96 * (h + 1) - f)
                        nc.scalar.dma_start(
                            out=slab_in[BLK * h + 192 + (f - 96 * h):
                                        BLK * h + 192 + (f - 96 * h) + nrow, :],
                            in_=v_sb1[r:r + nrow, :])
                        r += nrow

                # qk slices: matmuls + rank-1 LN corr + type emb, then
                # rope / tails -> slab.  Tails first (trig not needed).
                # The rope rot matmul for slice s is issued after slice
                # s+1's matmuls so the PE never waits on the DVE copy.
                def qk_slice(s):
                    qk_ps = p1qk.tile([128, TPC], F32, tag="qkps")
                    for k in range(KD):
                        nc.tensor.matmul(qk_ps[:],
                                         wqk[k][:, 128 * s:128 * (s + 1)],
                                         xs[k][:], start=(k == 0), stop=False)
                    nc.tensor.matmul(qk_ps[:],
                                     csqk_sb[:, 128 * s:128 * (s + 1)],
                                     be1b[:], start=False, stop=False)
                    nc.tensor.matmul(qk_ps[:],
                                     teq_sb[:, 128 * s:128 * (s + 1)],
                                     oh_q[:], start=False, stop=False)
                    nc.tensor.matmul(qk_ps[:],
                                     tek_sb[:, 128 * s:128 * (s + 1)],
                                     oh_k[:], start=False, stop=True)
                    return qk_ps

                def qk_tail(s, qk_ps):
                    tl = p1t.tile([128, TPC], BF16, tag="tail")
                    sc = float(SCALE) if s < 10 else 1.0
                    with nc.allow_low_precision(reason="bf16 payload"):
                        nc.vector.tensor_scalar(out=tl[:], in0=qk_ps[:],
                                                scalar1=sc, scalar2=None,
                                                op0=alu.mult)
                    base = 64 if s < 10 else 160
                    for j in range(4):
                        h = 4 * (s % 2) + j
                        nc.scalar.dma_start(
                            out=slab_in[BLK * h + base:BLK * h + base + 32, :],
                            in_=tl[32 * j:32 * (j + 1), :])

                def rope_start(s, qk_ps):
                    rsb = p1t.tile([128, TPC], F32R, tag="rsb")
                    nc.vector.tensor_copy(out=rsb[:], in_=qk_ps[:])
                    t1 = p1t.tile([128, TPC], F32, tag="rt1")
                    nc.vector.tensor_tensor(out=t1[:],
                                            in0=rsb[:].bitcast(F32),
                                            in1=c_t[:], op=alu.mult)
                    return (s, rsb, t1)

                def rope_finish(pend):
                    s, rsb, t1 = pend
                    rot = p1rot.tile([128, TPC], F32, tag="rot")
                    nc.tensor.matmul(rot[:], r128_sb[:], rsb[:],
                                     start=True, stop=True)
                    t2 = p1t.tile([128, TPC], F32, tag="rt2")
                    nc.vector.tensor_tensor(out=t2[:], in0=rot[:],
                                            in1=s_t[:], op=alu.mult)
                    qkr = p1t.tile([128, TPC], BF16, tag="qkr")
                    with nc.allow_low_precision(reason="bf16 payload"):
                        nc.vector.tensor_tensor(out=qkr[:], in0=t1[:],
                                                in1=t2[:], op=alu.add)
                    nc.scalar.dma_start(
                        out=slab_in[BLK * s + 0:BLK * s + 64, :],
                        in_=qkr[0:64, :])
                    nc.scalar.dma_start(
                        out=slab_in[BLK * s + 96:BLK * s + 160, :],
                        in_=qkr[64:128, :])

                pend = None
                for s in [8, 9, 10, 11] + list(range(8)):
                    qk_ps = qk_slice(s)
                    if pend is not None:
                        rope_finish(pend)
                        pend = None
                    if s < 8:
                        pend = rope_start(s, qk_ps)
                    else:
                        qk_tail(s, qk_ps)
                for sv in range(KD):
                    v_slice(sv)
                    if pend is not None:
                        rope_finish(pend)
                        pend = None

            nc.gpsimd.collective_compute(
                "AllToAll", mybir.AluOpType.bypass,
                replica_groups=[list(range(N_CORES))],
                ins=[slab_in[:].opt()],
                outs=[slab_out[:].opt()])

            # ================= PHASE 2 =================
            with contextlib.ExitStack() as p2:
                p2w = p2.enter_context(tc.tile_pool(name="p2w", bufs=1))
                p2t = p2.enter_context(tc.tile_pool(name="p2t", bufs=3))
                p2ps = p2.enter_context(
                    tc.tile_pool(name="p2ps", bufs=2, space="PSUM"))
                p2o = p2.enter_context(
                    tc.tile_pool(name="p2o", bufs=2, space="PSUM"))

                masks_sb = p2w.tile([128, 4 * 512], BF16, tag="masks")
                nc.sync.dma_start(out=masks_sb[:], in_=masks_d[:])
                id96_sb = p2w.tile([96, 96], BF16, tag="id96")
                nc.sync.dma_start(out=id96_sb[:], in_=id96_d[:])
                p2tr = p2.enter_context(
                    tc.tile_pool(name="p2tr", bufs=1, space="PSUM"))

                def softmax_tail(o_ps, j):
                    """Normalize finished unit j straight out of PSUM."""
                    rec = p2t.tile([1, 512], F32R, tag="rec", name="rec")
                    with nc.allow_low_precision(reason="softmax recip"):
                        nc.vector.reciprocal(out=rec[:], in_=o_ps[96:97, :])
                    rb = p2tr.tile([96, 512], F32, tag="rb", name="rb")
                    nc.tensor.matmul(rb[:], ones_sb[:, 0:96], rec[:],
                                     start=True, stop=True)
                    rb_sb = p2t.tile([96, 512], F32, tag="rbsb", name="rbsb")
                    nc.vector.tensor_copy(out=rb_sb[:], in_=rb[:])
                    onrm = p2t.tile([96, 512], BF16, tag="onrm", name="onrm")
                    with nc.allow_low_precision(reason="bf16 payload"):
                        nc.vector.tensor_tensor(out=onrm[:],
                                                in0=o_ps[0:96, :],
                                                in1=rb_sb[:], op=alu.mult)
                    nc.scalar.dma_start(
                        out=slab2_in[96 * j:96 * (j + 1), :], in_=onrm[:])

                # pend = (e_sb, o_ps, kt0, last_kt, j) -> two deferred o-mms
                def flush(pend, v_sb):
                    e_sb, o_ps, kt0, last, j = pend
                    for half in range(2):
                        kt = kt0 + half
                        nc.tensor.matmul(
                            o_ps[:], v_sb[:, kt, :],
                            e_sb[:, 512 * half:512 * (half + 1)],
                            start=(kt == 0), stop=(kt == last))
                    if kt0 + 1 == last:
                        softmax_tail(o_ps, j)

                for bb_ in range(2):
                    qT = p2w.tile([96, 2048], BF16, tag=f"qT{bb_}")
                    kT = p2w.tile([96, 2048], BF16, tag=f"kT{bb_}")
                    vF = p2w.tile([96, 2048], BF16, tag=f"vF{bb_}")
                    v_sb = p2w.tile([128, 16, 97], BF16, tag=f"v{bb_}")
                    nc.vector.memset(v_sb[:, :, 96:97], 1.0)
                    for u in range(4):
                        blk = BLK * (4 * bb_ + u)
                        nc.sync.dma_start(
                            out=qT[:, 512 * u:512 * (u + 1)],
                            in_=slab_out[blk + 0:blk + 96, :])
                        nc.sync.dma_start(
                            out=kT[:, 512 * u:512 * (u + 1)],
                            in_=slab_out[blk + 96:blk + 192, :])
                        nc.sync.dma_start(
                            out=vF[:, 512 * u:512 * (u + 1)],
                            in_=slab_out[blk + 192:blk + 288, :])

                    pend = None
                    for Q in range(NSUP):
                        # transpose the 4 new v k-tiles this Q unlocks
                        for kt in range(4 * Q, 4 * Q + 4):
                            vt_ps = p2tr.tile([128, 96], BF16, tag="vtps",
                                              name="vtps")
                            nc.tensor.transpose(
                                vt_ps[:], vF[:, 128 * kt:128 * (kt + 1)],
                                id96_sb[:])
                            nc.vector.tensor_copy(out=v_sb[:, kt, 0:96],
                                                  in_=vt_ps[:])
                        o_ps = p2o.tile([97, 512], F32, tag="ops", name="ops")
                        npair = 2 * Q + 2
                        for pr in range(npair):
                            s_ps = p2ps.tile([128, 1024], F32, tag="sps",
                                             name="sps")
                            for half in range(2):
                                kt = 2 * pr + half
                                nc.tensor.matmul(
                                    s_ps[:, 512 * half:512 * (half + 1)],
                                    kT[:, 128 * kt:128 * (kt + 1)],
                                    qT[:, 512 * Q:512 * (Q + 1)],
                                    start=True, stop=True)
                            if pend is not None:
                                flush(pend, v_sb)
                            e_sb = p2t.tile([128, 1024], BF16, tag="esb",
                                            name="esb")
                            with nc.allow_low_precision(reason="bf16 probs"):
                                nc.scalar.activation(out=e_sb[:], in_=s_ps[:],
                                                     func=act.Exp)
                            for half in range(2):
                                kt = 2 * pr + half
                                dj = kt - 4 * Q
                                if dj >= 0:
                                    eh = e_sb[:, 512 * half:512 * (half + 1)]
                                    nc.vector.tensor_tensor(
                                        out=eh, in0=eh,
                                        in1=masks_sb[:, 512 * dj:512 * (dj + 1)],
                                        op=alu.mult)
                            pend = (e_sb, o_ps, 2 * pr, 4 * Q + 3,
                                    4 * bb_ + Q)
                    flush(pend, v_sb)

            nc.gpsimd.collective_compute(
                "AllToAll", mybir.AluOpType.bypass,
                replica_groups=[list(range(N_CORES))],
                ins=[slab2_in[:].opt()], outs=[slab2_out[:].opt()])

            # ================= PHASE 3 =================
            with contextlib.ExitStack() as p3:
                p3w = p3.enter_context(tc.tile_pool(name="p3w", bufs=1))
                p3t = p3.enter_context(tc.tile_pool(name="p3t", bufs=2))
                p3ps = p3.enter_context(
                    tc.tile_pool(name="p3ps", bufs=2, space="PSUM"))

                cs1_sb = p3w.tile([1, 2 * DFF], BF16, tag="cs1")
                nc.sync.dma_start(out=cs1_sb[:], in_=cs1_d[:])

                x2 = []
                x2r = []
                for k in range(KD):
                    o_sb = p3t.tile([128, TPC], BF16, tag="osb")
                    nc.sync.dma_start(out=o_sb[:],
                                      in_=slab2_out[128 * k:128 * (k + 1), :])
                    t = p3w.tile([128, TPC], F32, tag=f"x2_{k}")
                    nc.vector.tensor_tensor(out=t[:], in0=o_sb[:],
                                            in1=xT[k][:], op=alu.add)
                    x2.append(t)
                    tr = p3w.tile([128, TPC], F32R, tag=f"x2r{k}")
                    nc.vector.tensor_copy(out=tr[:], in_=t[:])
                    x2r.append(tr)

                ab2, be2b = layernorm_fold(p3w, x2r, "l2")
                xs2 = []
                for k in range(KD):
                    t = p3w.tile([128, TPC], BF16, tag=f"xs2_{k}")
                    with nc.allow_low_precision(reason="bf16 activations"):
                        nc.vector.tensor_tensor(out=t[:],
                                                in0=x2r[k][:].bitcast(F32),
                                                in1=ab2[:], op=alu.mult)
                    xs2.append(t)

                # fused fc1 + silu + fc2 stream over 16 dff chunks.
                # fc2 for chunk i-1 issues after chunk i's fc1 matmuls so
                # the PE never waits on the silu/mult chain.
                with tc.tile_pool(name="p3f", bufs=1, space="PSUM") as p3f:
                    ff_ps = [p3f.tile([128, TPC], F32, tag=f"ff{d}",
                                      name=f"ff{d}")
                             for d in range(KD)]

                    def fc2(i, sw):
                        for d in range(KD):
                            nc.tensor.matmul(ff_ps[d][:],
                                             w2[i][:, 128 * d:128 * (d + 1)],
                                             sw[:],
                                             start=(i == 0), stop=(i == 15))

                    sw_pend = None
                    for i in range(16):
                        a_ps = p3ps.tile([128, TPC], F32, tag="hps",
                                         name="aps")
                        for k in range(KD):
                            nc.tensor.matmul(
                                a_ps[:],
                                w1[k][:, 128 * i:128 * (i + 1)],
                                xs2[k][:], start=(k == 0), stop=False)
                        nc.tensor.matmul(
                            a_ps[:], cs1_sb[:, 128 * i:128 * (i + 1)],
                            be2b[:], start=False, stop=True)
                        g_ps = p3ps.tile([128, TPC], F32, tag="hps",
                                         name="gps")
                        for k in range(KD):
                            nc.tensor.matmul(
                                g_ps[:],
                                w1[k][:, 2048 + 128 * i:2048 + 128 * (i + 1)],
                                xs2[k][:], start=(k == 0), stop=False)
                        nc.tensor.matmul(
                            g_ps[:],
                            cs1_sb[:, 2048 + 128 * i:2048 + 128 * (i + 1)],
                            be2b[:], start=False, stop=True)
                        if sw_pend is not None:
                            fc2(i - 1, sw_pend)
                        a_sb = p3t.tile([128, TPC], F32, tag="asb")
                        nc.vector.tensor_scalar(
                            out=a_sb[:], in0=a_ps[:],
                            scalar1=b1a_sb[:, i:i + 1],
                            scalar2=None, op0=alu.add)
                        sil = p3t.tile([128, TPC], F32, tag="sil")
                        nc.scalar.activation(
                            out=sil[:], in_=g_ps[:], func=act.Silu,
                            bias=b1g_sb[:, i:i + 1])
                        sw = p3t.tile([128, TPC], BF16, tag="sw")
                        with nc.allow_low_precision(reason="bf16 ffn acts"):
                            nc.vector.tensor_tensor(out=sw[:], in0=sil[:],
                                                    in1=a_sb[:], op=alu.mult)
                        sw_pend = sw
                    fc2(15, sw_pend)
                    for d in range(KD):
                        o = p3t.tile([128, TPC], F32, tag="oout")
                        nc.vector.scalar_tensor_tensor(
                            out=o[:], in0=ff_ps[d][:],
                            scalar=bf2_sb[:, d:d + 1], in1=x2[d][:],
                            op0=alu.add, op1=alu.add)
                        nc.sync.dma_start(
                            out=outT_d[128 * d:128 * (d + 1), :], in_=o[:])

    nc.compile()
    _prog_cache[key] = nc
    return nc


def _host_inputs(x_type, x_value, seq_order, W_attn, type_emb, g1, b1, g2, b2,
                 W_fc1, b_fc1, W_fc2, b_fc2):
    f32 = np.float32
    bf16 = mybir.dt.np(mybir.dt.bfloat16)
    x_type = np.asarray(x_type)
    seq_order = np.asarray(seq_order)
    x_value = np.asarray(x_value, dtype=f32)
    W_attn = np.asarray(W_attn, dtype=f32)
    type_emb = np.asarray(type_emb, dtype=f32)
    W_fc1 = np.asarray(W_fc1, dtype=f32)
    W_fc2 = np.asarray(W_fc2, dtype=f32)
    g1 = np.asarray(g1, f32); b1 = np.asarray(b1, f32)
    g2 = np.asarray(g2, f32); b2 = np.asarray(b2, f32)
    b_fc1 = np.asarray(b_fc1, f32); b_fc2 = np.asarray(b_fc2, f32)

    # ---- qk weights: permute, fold g1; fold b1 bias into type emb ----
    w_perm = W_attn[:, :1536][:, QK_PERM]
    wqk_bf = (w_perm * g1[:, None]).astype(bf16)
    csqk = wqk_bf.astype(f32).sum(axis=0, keepdims=True).astype(bf16)
    bias_qk = (b1 @ w_perm).astype(f32)          # (1536,)
    te_full = type_emb[:, QK_PERM]               # (16, 1536)
    q_origin = QK_PERM < 768
    te_q = np.where(q_origin[None, :], te_full + bias_qk[None, :], 0.0)
    te_k = np.where(~q_origin[None, :], te_full + bias_qk[None, :], 0.0)

    # ---- v weights: fold g1; beta/bias correction rows ----
    wv_bf = (W_attn[:, 1536:] * g1[:, None]).astype(bf16)
    csv = np.stack([
        wv_bf.astype(f32).sum(axis=0),           # colsum row (x beta)
        (b1 @ W_attn[:, 1536:]).astype(f32),     # bias row (x ones)
    ]).astype(bf16)

    # ---- ffn weights: fold g2; fold b2 into fc1 bias ----
    w1_bf = (W_fc1 * g2[:, None]).astype(bf16)
    cs1 = w1_bf.astype(f32).sum(axis=0, keepdims=True).astype(bf16)
    bias_fc1 = (b_fc1 + b2 @ W_fc1).astype(f32)  # (4096,)
    w2_bf = W_fc2.astype(bf16)

    invf16 = (1.0 / THETA ** (np.arange(0, DR, 2, dtype=f32) / DR)).astype(f32)
    invf_col = invf16[(np.arange(128) % 32) // 2].reshape(128, 1)

    # masks: block (128k x 512q), mask[kk, qq] = 1 if qq >= kk + 128*dj
    kk = np.arange(128)[:, None]
    qq = np.arange(512)[None, :]
    masks = np.concatenate(
        [(qq >= kk + 128 * dj).astype(f32) for dj in range(4)],
        axis=1).astype(bf16)

    # rot lhsT: lhsT[k, m] = P[m, k];  P[2i, 2i+1] = -1, P[2i+1, 2i] = +1
    R = np.zeros((128, 128), f32)
    for i in range(64):
        R[2 * i + 1, 2 * i] = -1.0
        R[2 * i, 2 * i + 1] = 1.0
    B4m = np.zeros((4, 128), f32)
    B4m[0, 0:32] = 1.0; B4m[1, 32:64] = 1.0
    B4m[2, 64:96] = 1.0; B4m[3, 96:128] = 1.0

    common = {
        "Wqk": wqk_bf, "Wv": wv_bf,
        "te_q": te_q.astype(bf16), "te_k": te_k.astype(bf16),
        "csqk": csqk, "csv": csv,
        "invf": invf_col,
        "W1": w1_bf, "cs1": cs1, "W2": w2_bf,
        "b1a": bias_fc1[:2048].reshape(16, 128).T.copy(),
        "b1g": bias_fc1[2048:].reshape(16, 128).T.copy(),
        "bf2": b_fc2.reshape(6, 128).T.copy(),
        "masks": masks, "R128": R, "B4": B4m,
        "id96": np.eye(96, dtype=f32).astype(bf16),
        "ones128": np.ones((1, 128), f32),
        "onescol": np.ones((128, 1), f32),
        "iota16": np.arange(16, dtype=f32).reshape(16, 1),
        "epsc": np.full((1, 1), EPS, f32),
    }
    in_maps = []
    for c in range(N_CORES):
        b = c // 4
        t0 = 512 * (c % 4)
        m = dict(common)
        m["xT"] = np.ascontiguousarray(x_value[b, t0:t0 + TPC, :].T)
        m["qtype"] = x_type[b, t0:t0 + TPC].astype(f32).reshape(1, TPC)
        m["ktype"] = x_type[b, t0 + 1:t0 + TPC + 1].astype(f32).reshape(1, TPC)
        pos4 = np.stack([
            seq_order[0, b, t0:t0 + TPC],
            seq_order[1, b, t0:t0 + TPC],
            seq_order[0, b, t0 + 1:t0 + TPC + 1],
            seq_order[1, b, t0 + 1:t0 + TPC + 1],
        ]).astype(f32)
        m["pos4"] = pos4
        in_maps.append(m)
    return in_maps


def kernel(**inputs):
    nc = build_program()
    in_maps = _host_inputs(**inputs)
    res = run_bass_kernel_spmd(nc, in_maps, list(range(N_CORES)), trace=False)
    out = np.empty((B, T, D), np.float32)
    for c in range(N_CORES):
        b = c // 4
        t0 = 512 * (c % 4)
        out[b, t0:t0 + TPC, :] = res.results[c]["outT"].T
    return out
